# revision 14
# baseline (speedup 1.0000x reference)
"""Talking-heads attention kernel for Trainium2 (8 NeuronCores, SPMD).

Problem: B=4, N=1024, C=768, H=12, D=64 talking-heads attention.
Sharding: 8 cores = (batch b in 0..3) x (query half in 0..1); each core
computes attention for 512 queries of one batch element (K/V over the
full 1024 keys of that element). No collectives needed.

Per-core pipeline (all layouts chosen so every matmul contracts over
partitions at full width where it matters):
  1. x -> SBUF, PE-transpose to xT [c=768(part-chunks), n=1024].
  2. QKV projections: QT [768, 513(pad)], KT [768, 1024] (transposed
     world, d on partitions) and V [1024, 768] (natural world, m on
     partitions, bf16).
  3. Per head h and query-chunk: S = QT[h].T @ KT[h]  [cn, 1024] psum.
  4. Shuffle-DMA S into Kronecker block layout [(h, n9)=108(+9 mask
     rows), grp, m] so the talking-heads PRE-mix becomes a single
     matmul with lhsT = [kron(w_pre.T, I9); kron(rowW, I9)] (the extra
     9 contraction rows fold the additive attn_mask in, pre-scaled by
     rowW[g] = sum_h w_pre[g,h]).
  5. exp on ACT with fused row-sum (no max subtraction needed: logits
     are bounded ~|1.5| for this problem), reciprocal + normalize.
  6. POST-mix with swapped operands: lhsT = P[:, mc*128:...] so the
     output comes out TRANSPOSED [m, (g, n9)] - exactly what AV needs.
  7. AV: lhsT = V[mc, g-cols], rhs = PT strided slice -> OT [768, n].
  8. proj: lhsT = OT chunks, rhs = wprojT -> out rows, + bias, DMA out.

float32 data everywhere except P/PT/V/kron_post (bf16, error-tolerant);
matmuls with free dim >= 256 are issued as float32r (1 cycle/row vs 4
for plain fp32 on TRN2).
"""

import numpy as np

import concourse.bass as bass
import concourse.mybir as mybir
import concourse.tile as tile
from concourse import bacc
from concourse.bass_utils import run_bass_kernel_spmd
from concourse.masks import make_identity

B, N, C = 4, 1024, 768
H, D = 12, 64
SCALE = np.float32(D**-0.5)
NQ = 512  # queries per core
NS = 9  # queries per Kron sub-block
NGRP = 57  # groups of NS (513 padded queries)
NQP = NGRP * NS  # 513
CHUNK_GRPS = [12, 12, 12, 12, 9]  # groups per processing chunk
KC = C // 128  # 6 contraction chunks of 128
MT = N // 128  # 8 key/m chunks of 128

F32 = mybir.dt.float32
F32R = mybir.dt.float32r
BF16 = mybir.dt.bfloat16

USE_F32R = True


def _r(ap):
    """Operand tiles are already float32r; kept as a hook point."""
    return ap


def build_nc(debug=False):
    nc = bacc.Bacc(None, target_bir_lowering=False)

    x_d = nc.declare_dram_parameter("x", [N, C], F32, isOutput=False)
    mask_d = nc.declare_dram_parameter("mask", [NQP, N], F32R, isOutput=False)
    wqkT_d = nc.declare_dram_parameter("wqkT", [C, 2 * C], F32R, isOutput=False)
    wvT_d = nc.declare_dram_parameter("wvT", [C, C], F32R, isOutput=False)
    wpT_d = nc.declare_dram_parameter("wpT", [C, C], F32R, isOutput=False)
    bias_d = nc.declare_dram_parameter("biasp", [C], F32, isOutput=False)
    kpre_d = nc.declare_dram_parameter("kron_pre", [117, 108], F32R, isOutput=False)
    kpost_d = nc.declare_dram_parameter("kron_post", [108, 108], BF16, isOutput=False)
    qz_d = nc.declare_dram_parameter("qzero", [128, KC], F32R, isOutput=False)
    out_d = nc.declare_dram_parameter("out", [NQ, C], F32, isOutput=True)
    dbg = None
    if debug:
        dbg = {
            "dbg_qt": nc.declare_dram_parameter("dbg_qt", [128, KC, NQP], F32, isOutput=True),
            "dbg_kt": nc.declare_dram_parameter("dbg_kt", [128, KC, N], F32, isOutput=True),
            "dbg_v": nc.declare_dram_parameter("dbg_v", [128, MT, C], F32, isOutput=True),
            "dbg_sk": nc.declare_dram_parameter("dbg_sk", [128, 12, N], F32, isOutput=True),
            "dbg_pe": nc.declare_dram_parameter("dbg_pe", [108, N], F32, isOutput=True),
            "dbg_pb": nc.declare_dram_parameter("dbg_pb", [108, N], F32, isOutput=True),
            "dbg_pt": nc.declare_dram_parameter("dbg_pt", [128, MT, 12, 108], F32, isOutput=True),
            "dbg_ot": nc.declare_dram_parameter("dbg_ot", [128, KC, 108], F32, isOutput=True),
        }

    with tile.TileContext(nc) as tc:
        build_body(nc, tc, x_d, mask_d, wqkT_d, wvT_d, wpT_d, bias_d,
                   kpre_d, kpost_d, qz_d, out_d, dbg=dbg)
    nc.compile()
    return nc


def build_body(nc, tc, x_d, mask_d, wqkT_d, wvT_d, wpT_d, bias_d,
               kpre_d, kpost_d, qz_d, out_d, dbg=None):
    from contextlib import ExitStack

    # ---------------- persistent tiles ----------------
    with ExitStack() as ctx:
        singles = ctx.enter_context(tc.tile_pool(name="singles", bufs=1))

        ident = singles.tile([128, 128], F32)
        make_identity(nc, ident)

        kpre_sb = singles.tile([117, 108], F32R)
        nc.sync.dma_start(out=kpre_sb, in_=kpre_d[:, :])
        kpost_sb = singles.tile([108, 108], BF16)
        nc.sync.dma_start(out=kpost_sb, in_=kpost_d[:, :])

        wpT_sb = singles.tile([128, KC, C], F32R)
        nc.sync.dma_start(out=wpT_sb, in_=wpT_d.rearrange("(k p) c -> p k c", p=128))

        bias_sb = singles.tile([128, C], F32)
        bap = bias_d.ap()
        bias_bc = bass.AP(tensor=bap.tensor, offset=bap.offset,
                          ap=[[0, 128]] + list(bap.ap))
        nc.sync.dma_start(out=bias_sb, in_=bias_bc)

        # outputs of phase 1 (persist through phase 2/3)
        qt_sb = singles.tile([128, KC, NQP], F32R)  # QT padded to 513 cols
        kt_sb = singles.tile([128, KC, N], F32R)
        v_sb = singles.tile([128, MT, C], BF16)

        # ---------------- phase 1: x^T and projections ----------------
        with ExitStack() as p1:
            xw_pool = p1.enter_context(tc.tile_pool(name="xw", bufs=1))
            ps_t = p1.enter_context(tc.tile_pool(name="ps_t", bufs=4, space="PSUM"))
            ps_qkv = p1.enter_context(tc.tile_pool(name="ps_qkv", bufs=4, space="PSUM"))

            x_sb = xw_pool.tile([128, MT, C], F32)
            nc.sync.dma_start(out=x_sb, in_=x_d.rearrange("(t p) c -> p t c", p=128))
            wqkT_sb = xw_pool.tile([128, KC, 2 * C], F32R)
            nc.sync.dma_start(out=wqkT_sb,
                              in_=wqkT_d.rearrange("(k p) c -> p k c", p=128))
            wvT_sb = xw_pool.tile([128, KC, C], F32R)
            nc.sync.dma_start(out=wvT_sb,
                              in_=wvT_d.rearrange("(k p) c -> p k c", p=128))

            xt_sb = xw_pool.tile([128, KC, N], F32R)
            nc.sync.dma_start(out=qt_sb[:, :, NQ],
                              in_=qz_d[:, :])
            for t in range(MT):
                for k in range(KC):
                    pt = ps_t.tile([128, 128], F32, tag="pt")
                    nc.tensor.transpose(pt, x_sb[:, t, k * 128:(k + 1) * 128], ident)
                    nc.any.tensor_copy(out=xt_sb[:, k, t * 128:(t + 1) * 128], in_=pt)

            # QT (our query half only; host passes x so that cols half*512.. are
            # the queries -> actually host passes full x; the q columns used are
            # selected by host via the mask/out slicing. Here we compute Q^T for
            # columns QOFF..QOFF+512 of n. QOFF is baked by the host into wqkT?
            # No: we pass q_half as a separate input-free trick: host rolls x so
            # queries are always cols 0..512. See host prep in kernel().
            for oc in range(KC):
                pq = ps_qkv.tile([128, NQ], F32, tag="pq")
                for k in range(KC):
                    nc.tensor.matmul(pq, _r(wqkT_sb[:, k, oc * 128:(oc + 1) * 128]),
                                     _r(xt_sb[:, k, 0:NQ]),
                                     start=(k == 0), stop=(k == KC - 1))
                nc.any.tensor_copy(out=qt_sb[:, oc, 0:NQ], in_=pq)
            # KT full n
            for oc in range(KC):
                for nh in range(2):
                    pk = ps_qkv.tile([128, NQ], F32, tag="pq")
                    for k in range(KC):
                        nc.tensor.matmul(
                            pk,
                            _r(wqkT_sb[:, k, C + oc * 128:C + (oc + 1) * 128]),
                            _r(xt_sb[:, k, nh * NQ:(nh + 1) * NQ]),
                            start=(k == 0), stop=(k == KC - 1))
                    nc.any.tensor_copy(out=kt_sb[:, oc, nh * NQ:(nh + 1) * NQ], in_=pk)
            # V natural [m, o] in bf16
            for t in range(MT):
                for f, fw in ((0, NQ), (1, 256)):
                    pv = ps_qkv.tile([128, NQ], F32, tag="pq")
                    for k in range(KC):
                        nc.tensor.matmul(pv[:, :fw],
                                         _r(xt_sb[:, k, t * 128:(t + 1) * 128]),
                                         _r(wvT_sb[:, k, f * NQ:f * NQ + fw]),
                                         start=(k == 0), stop=(k == KC - 1))
                    nc.any.tensor_copy(out=v_sb[:, t, f * NQ:f * NQ + fw],
                                       in_=pv[:, :fw])

        if dbg is not None:
            nc.sync.dma_start(out=dbg["dbg_qt"][:, :, :], in_=qt_sb.bitcast(F32))
            nc.sync.dma_start(out=dbg["dbg_kt"][:, :, :], in_=kt_sb.bitcast(F32))
            nc.gpsimd.dma_start(out=dbg["dbg_v"][:, :, :], in_=v_sb)

        # ---------------- phase 2: attention ----------------
        with ExitStack() as p2:
            sn_pool = p2.enter_context(tc.tile_pool(name="s_nat", bufs=1))
            sk_pool = p2.enter_context(tc.tile_pool(name="s_kron", bufs=2))
            p_pool = p2.enter_context(tc.tile_pool(name="probs", bufs=2))
            pt_pool = p2.enter_context(tc.tile_pool(name="pt", bufs=1))
            ot_pool = p2.enter_context(tc.tile_pool(name="ot", bufs=2))
            os_pool = p2.enter_context(tc.tile_pool(name="out_sb", bufs=1))
            ps_small = p2.enter_context(
                tc.tile_pool(name="ps_small", bufs=4, space="PSUM"))
            ps_mix = p2.enter_context(
                tc.tile_pool(name="ps_mix", bufs=2, space="PSUM"))

            for c, ngrp in enumerate(CHUNK_GRPS):
                cn = ngrp * NS
                n0 = c * 108
                # S per head into sn [(j s), h, m]; one plain DMA per group
                # then lands it as sk [(s h), j, m] (kron_pre rows are s*12+h)
                sk = [sk_pool.tile([128, ngrp, NQ], F32R, tag="sk",
                                   name=f"sk{mh}") for mh in range(2)]
                for mh in range(2):
                    nc.sync.dma_start(
                        out=sk[mh][108:117, 0:ngrp, :],
                        in_=mask_d[n0:n0 + cn, mh * NQ:(mh + 1) * NQ].rearrange(
                            "(j s) m -> s j m", s=NS))
                for mh in range(2):
                    sn = sn_pool.tile([108, H, NQ], F32R, tag="sn")
                    for h in range(H):
                        hp = (h % 2) * 64
                        hk = h // 2
                        ps_s = ps_small.tile([108, NQ], F32, tag="small")
                        nc.tensor.matmul(
                            ps_s[:cn, :],
                            _r(qt_sb[hp:hp + 64, hk, n0:n0 + cn]),
                            _r(kt_sb[hp:hp + 64, hk, mh * NQ:(mh + 1) * NQ]),
                            start=True, stop=True)
                        nc.any.tensor_copy(out=sn[:cn, h, :], in_=ps_s[:cn, :])
                    for j in range(ngrp):
                        nc.sync.dma_start(
                            out=sk[mh][0:108, j, :],
                            in_=sn[j * NS:(j + 1) * NS, :, :])

                if dbg is not None and c == 0:
                    for mh in range(2):
                        nc.sync.dma_start(
                            out=dbg["dbg_sk"][0:117, :, mh * NQ:(mh + 1) * NQ],
                            in_=sk[mh][0:117, :, :].bitcast(F32))
                for j in range(ngrp):
                    pm = ps_mix.tile([108, N], F32, tag="mix")
                    for mh in range(2):
                        nc.tensor.matmul(pm[:, mh * NQ:(mh + 1) * NQ],
                                         _r(kpre_sb), _r(sk[mh][0:117, j, :]),
                                         start=True, stop=True)
                    pe = p_pool.tile([108, N], F32, tag="pe")
                    zsum = p_pool.tile([108, 1], F32, tag="zs")
                    nc.scalar.activation(out=pe, in_=pm,
                                         func=mybir.ActivationFunctionType.Exp,
                                         accum_out=zsum)
                    rz = p_pool.tile([108, 1], F32, tag="rz")
                    nc.vector.reciprocal(out=rz, in_=zsum)
                    pb = p_pool.tile([108, N], BF16, tag="pb")
                    nc.vector.tensor_scalar_mul(out=pb, in0=pe, scalar1=rz)
                    if dbg is not None and c == 0 and j == 0:
                        nc.sync.dma_start(out=dbg["dbg_pe"][:, :], in_=pe)
                        nc.gpsimd.dma_start(out=dbg["dbg_pb"][:, :], in_=pb)

                    if j == 0:
                        ptc = pt_pool.tile([128, MT, ngrp, 108], BF16, tag="ptc")
                    for mc in range(MT):
                        pp = ps_small.tile([128, 108], F32, tag="small")
                        nc.tensor.matmul(pp, pb[:, mc * 128:(mc + 1) * 128],
                                         kpost_sb, start=True, stop=True)
                        nc.any.tensor_copy(out=ptc[:, mc, j, :], in_=pp)

                if dbg is not None and c == 0:
                    nc.gpsimd.dma_start(out=dbg["dbg_pt"][:, :, :, :], in_=ptc)
                # AV per output head
                otc = ot_pool.tile([128, KC, 108], F32R, tag="otc")
                for g in range(H):
                    pav = ps_small.tile([64, 108], F32, tag="small")
                    for mc in range(MT):
                        nc.tensor.matmul(
                            pav[:, :cn],
                            v_sb[:, mc, g * 64:(g + 1) * 64],
                            ptc[:, mc, 0:ngrp, g * NS:(g + 1) * NS],
                            start=(mc == 0), stop=(mc == MT - 1))
                    gp = (g % 2) * 64
                    nc.any.tensor_copy(out=otc[gp:gp + 64, g // 2, :cn],
                                       in_=pav[:, :cn])

                if dbg is not None and c == 0:
                    nc.sync.dma_start(out=dbg["dbg_ot"][:, :, :], in_=otc.bitcast(F32))
                # proj + bias + out
                po = ps_mix.tile([108, C], F32, tag="mix")
                for f, fw in ((0, NQ), (1, 256)):
                    for k in range(KC):
                        nc.tensor.matmul(po[:cn, f * NQ:f * NQ + fw],
                                         _r(otc[:, k, :cn]),
                                         _r(wpT_sb[:, k, f * NQ:f * NQ + fw]),
                                         start=(k == 0), stop=(k == KC - 1))
                osb = os_pool.tile([108, C], F32, tag="osb")
                nc.vector.tensor_add(out=osb[:cn, :], in0=po[:cn, :],
                                     in1=bias_sb[:cn, :])
                rows = min(NQ - n0, cn)
                nc.sync.dma_start(out=out_d[n0:n0 + rows, :], in_=osb[:rows, :])


_NC_CACHE = {}


def _get_nc():
    if "nc" not in _NC_CACHE:
        _NC_CACHE["nc"] = build_nc()
    return _NC_CACHE["nc"]


def kernel(x, attn_mask, w_qkv, w_proj, b_proj, w_pre, w_post):
    x = np.ascontiguousarray(np.asarray(x, dtype=np.float32))
    attn_mask = np.ascontiguousarray(np.asarray(attn_mask, dtype=np.float32))
    w_qkv = np.asarray(w_qkv, dtype=np.float32)
    w_proj = np.asarray(w_proj, dtype=np.float32)
    b_proj = np.asarray(b_proj, dtype=np.float32)
    w_pre = np.asarray(w_pre, dtype=np.float32)
    w_post = np.asarray(w_post, dtype=np.float32)

    wqT = np.ascontiguousarray((w_qkv[:C] * SCALE).T)
    wkT = np.ascontiguousarray(w_qkv[C:2 * C].T)
    wqkT = np.ascontiguousarray(np.concatenate([wqT, wkT], axis=1))  # [768, 1536]
    wvT = np.ascontiguousarray(w_qkv[2 * C:].T)
    wpT = np.ascontiguousarray(w_proj.T)
    eye = np.eye(NS, dtype=np.float32)
    rowW = w_pre.sum(axis=1).astype(np.float32)
    kron_pre = np.zeros((117, 108), dtype=np.float32)
    for s in range(NS):
        for h in range(H):
            kron_pre[s * H + h, s::NS] = w_pre[:, h]  # cols (g, s'=s)
        kron_pre[108 + s, s::NS] = rowW
    kron_post = np.kron(w_post.T.astype(np.float32), eye)  # [108, 108]
    import ml_dtypes
    kron_post_bf = kron_post.astype(ml_dtypes.bfloat16)

    in_maps = []
    for core in range(8):
        b, half = core // 2, core % 2
        q0 = half * NQ
        mk = np.zeros((NQP, N), dtype=np.float32)
        # roll x so the query half is always rows 0..512; keys/values come
        # out in the same rolled order, so the mask columns roll too
        # (softmax/AV are permutation-invariant over keys).
        if half == 0:
            xb = x[b]
            mk[:NQ] = attn_mask[b, q0:q0 + NQ]
        else:
            xb = np.ascontiguousarray(np.roll(x[b], -NQ, axis=0))
            mk[:NQ] = np.roll(attn_mask[b, q0:q0 + NQ], -NQ, axis=1)
        in_maps.append({
            "x": xb,
            "mask": mk,
            "wqkT": wqkT,
            "wvT": wvT,
            "wpT": wpT,
            "biasp": b_proj,
            "kron_pre": kron_pre,
            "kron_post": kron_post_bf,
            "qzero": np.zeros((128, KC), dtype=np.float32),
        })

    nc = _get_nc()
    _NC_CACHE["last_in_maps"] = in_maps
    res = run_bass_kernel_spmd(nc, in_maps, core_ids=list(range(8)))
    out = np.zeros((B, N, C), dtype=np.float32)
    for core in range(8):
        b, half = core // 2, core % 2
        out[b, half * NQ:(half + 1) * NQ] = res.results[core]["out"]
    return out
